# revision 14
# baseline (speedup 1.0000x reference)
"""GPTQ 4-bit dequant + linear (x @ W.T + bias) on 8 Trainium2 NeuronCores.

Problem shapes (hardcoded):
  x       [4, 2048, 4096] f32   -> host-tiled to bf16*2^4 [64, 128, 32, 128]
  qweight [16384, 512]    i32   (8x 4-bit nibbles per int32 along K)
  qzeros  [16384, 4]      i32
  scales  [16384, 32]     f32
  bias    [16384]         f32
  out     [4, 2048, 16384] f32

Sharding: column-parallel over out_features. Each of the 8 cores gets a
2048-row slab of qweight/qzeros/scales/bias; x replicated; outputs are
concatenated on the host along the feature axis.

Mixed precision: the last NFP8 of the 32 k-chunks run as fp8e4
DoubleRow matmuls (2 k-chunks per MM at ~1.4x bf16 throughput); the
rest stay bf16. The whole kernel computes at scale 2^12 (x*2^4 host,
w*2^8 dequant) so both paths accumulate in one PSUM bank; the fused
DVE drain rescales by 2^-12 and adds bias. fp8 on 6/32 of K measures
rel_err ~1.75e-2 vs the 2e-2 gate.

Engine/queue separation (strict-FIFO queues must not host ops that
wait on long dependency chains in front of compute):
  sync   = qw/qz/sc loads + the per-chunk wT xbar transposes
  gpsimd = all x tile loads (incl. casting fp8 slice loads), output
           stores, bias broadcast  (SWDGE: queue cost is issue-only)
  scalar = ACT dequant groups + wT8 fp8 cast
  vector = nibble extract, DVE dequant groups, PSUM drains

Phase A dequant emits 16 n-chunks (128 rows each); every 4 chunks
complete a 512-col "window" = one matmul n-block. PE idle during
Phase A is filled by a ROTATING prologue: window 0 is processed in
128-col sub-blocks per chunk as soon as each chunk's transpose lands
(tokens 0-3), then each window w runs one n-block for a fresh batch of
~8 token chunks streaming through the 5 xT buffers. Phase B reloads
those chunks to finish their remaining n-blocks, then streams the
rest.
"""
import sys

for _p in ("/opt/trn_rl_repo", "/root/.axon_site/_ro/trn_rl_repo"):
    if _p not in sys.path:
        sys.path.append(_p)

import numpy as np
import ml_dtypes
import concourse.bass as bass
import concourse.mybir as mybir
from concourse import tile, bacc
from concourse.bass_utils import run_bass_kernel_spmd

BF16 = mybir.dt.bfloat16
F32 = mybir.dt.float32
I32 = mybir.dt.int32
FP8 = mybir.dt.float8e4

B, S, K, N = 4, 2048, 4096, 16384
T = B * S                      # 8192 tokens
NCORES = 8
NS = N // NCORES               # 2048 out features per core
PACK = 8
GS = 128                       # quant group size
G = K // GS                    # 32 groups == 32 k-chunks
TCH = 128                      # tokens per chunk
KC = K // 128                  # 32 k-chunks
MMN = 512                      # matmul moving free dim (one PSUM bank of f32)
NBLK = NS // MMN               # 4
NCH = NS // 128                # 16 weight n-chunks
HALF = K // 2                  # dequant processed in 2 half-chunks
NFP8 = 6                       # trailing k-chunks on the fp8 DoubleRow path
KCB = KC - NFP8                # leading bf16 k-chunks
XSC = 16.0                     # x pre-scale (2^4, host side)
WSC = 256.0                    # w dequant scale (2^8, device side)
OSC = 1.0 / (XSC * WSC)        # drain rescale 2^-12

NSUB0 = 4                      # tokens processed per 128-col sub-block (win 0)
NROT = 8                       # rotation tokens per window (windows 0..3)

_LSR = mybir.AluOpType.logical_shift_right
_AND = mybir.AluOpType.bitwise_and
_SUB = mybir.AluOpType.subtract
_MUL = mybir.AluOpType.mult
_ADD = mybir.AluOpType.add
IDENT = mybir.ActivationFunctionType.Identity
DROW = mybir.MatmulPerfMode.DoubleRow

# engine per quant group: 24 ACT / 8 DVE
ENG32 = ['v' if g % 4 == 1 else 'a' for g in range(32)]


def _prologue_plan(nt):
    """(sub0 tokens, {window: [(token, block)]}, {token: remaining blocks})"""
    sub0 = list(range(min(NSUB0, nt)))
    remaining = {ti: list(range(NBLK)) for ti in range(nt)}
    for ti in sub0:
        remaining[ti].remove(0)
    rot = {w: [] for w in range(NBLK)}
    nxt = len(sub0)
    for w in range(NBLK):
        cnt = NROT - 1 if w == 0 else NROT
        for _ in range(cnt):
            if nxt >= nt:
                break
            rot[w].append((nxt, w))
            remaining[nxt].remove(w)
            nxt += 1
    return sub0, rot, remaining


def build(t_total: int = T):
    nt = t_total // TCH
    nc = bacc.Bacc("TRN2", target_bir_lowering=False, debug=False)
    xt_d = nc.dram_tensor("xt", [nt, 128, KC, TCH], BF16, kind="ExternalInput")
    qw_d = nc.dram_tensor("qw", [NS, K // PACK], I32, kind="ExternalInput")
    qz_d = nc.dram_tensor("qz", [NS, G // PACK], I32, kind="ExternalInput")
    sc_d = nc.dram_tensor("sc", [NS, G], F32, kind="ExternalInput")
    b_d = nc.dram_tensor("b", [NS], F32, kind="ExternalInput")
    out_d = nc.dram_tensor("out", [t_total, NS], F32, kind="ExternalOutput")

    sub0, rot, remaining = _prologue_plan(nt)

    with tile.TileContext(nc) as tc:
        with (
            tc.tile_pool(name="wtp", bufs=1) as wtpool,
            tc.tile_pool(name="consts", bufs=1) as cpool,
            tc.tile_pool(name="aload", bufs=2) as apool,
            tc.tile_pool(name="anib", bufs=2) as nibpool,
            tc.tile_pool(name="awch", bufs=2) as wchpool,
            tc.tile_pool(name="awt8", bufs=2) as w8pool,
            tc.tile_pool(name="bxt", bufs=5) as bxtpool,
            tc.tile_pool(name="bxq", bufs=4) as xqpool,
            tc.tile_pool(name="bout", bufs=2) as bopool,
            tc.tile_pool(name="ps", bufs=8, space=bass.MemorySpace.PSUM) as pspool,
        ):
            # persistent dequantized W.T * 256:
            #   bf16 [128 kk, 26 c, 2048 n] + fp8 [128 kk, 6 c, 2048 n]
            wT = wtpool.tile([128, KCB, NS], BF16)
            wT8 = wtpool.tile([128, NFP8, NS], FP8)

            # helpers -------------------------------------------------
            def load_x(ti):
                xT_t = bxtpool.tile([128, KC, TCH], BF16, name="xT", tag="xT")
                nc.gpsimd.dma_start(xT_t[:], xt_d[ti])
                return xT_t

            def load_xq(ti):
                # fp8 copy of the trailing NFP8 k-chunks via casting SWDGE
                xq_t = xqpool.tile([128, NFP8, TCH], FP8, name="xq", tag="xq")
                nc.gpsimd.dma_start(xq_t[:], xt_d[ti, :, KCB:KC, :])
                return xq_t

            def mm_cols(ps_t, xT_t, xq_t, n0, width):
                nsl = slice(n0, n0 + width)
                for c in range(KCB):
                    nc.tensor.matmul(
                        ps_t[:], xT_t[:, c, :], wT[:, c, nsl],
                        start=(c == 0), stop=False)
                for cp in range(0, NFP8, 2):
                    nc.tensor.matmul(
                        ps_t[:], xq_t[:, cp:cp + 2, :], wT8[:, cp:cp + 2, nsl],
                        start=False, stop=(cp == NFP8 - 2), perf_mode=DROW)

            def drain_store(ps_t, t0, n0, width):
                o_t = bopool.tile([128, width], F32, name="o_nb", tag="o_nb")
                # out = psum * 2^-12 + bias  (one fused DVE op)
                nc.vector.scalar_tensor_tensor(
                    out=o_t[:], in0=ps_t[:], scalar=OSC,
                    in1=bias_t[:, n0:n0 + width], op0=_MUL, op1=_ADD)
                nc.gpsimd.dma_start(out_d[t0:t0 + TCH, n0:n0 + width], o_t[:])

            def block(xT_t, xq_t, ti, n0, width):
                ps_t = pspool.tile([128, width], F32, name="psnb", tag="psnb")
                mm_cols(ps_t, xT_t, xq_t, n0, width)
                drain_store(ps_t, ti * TCH, n0, width)

            # stage the window-0 sub-block tokens
            xts = {}
            xqs = {}
            for ti in sub0:
                xts[ti] = load_x(ti)
                xqs[ti] = load_xq(ti)

            # bias broadcast to all 128 partitions: [128, 2048] bf16
            bias_t = cpool.tile([128, NS], BF16)
            b_row = b_d[:].rearrange("(o n) -> o n", o=1)
            b_bcast = bass.AP(tensor=b_row.tensor, offset=b_row.offset,
                              ap=[[0, 128], b_row.ap[1]])
            nc.gpsimd.dma_start(out=bias_t[:], in_=b_bcast)

            # ---- Phase A: dequantize weight slab, n-chunks of 128 rows
            for j in range(NCH):
                n0 = j * 128
                qw_t = apool.tile([128, K // PACK], I32)
                nc.sync.dma_start(qw_t[:], qw_d[n0:n0 + 128, :])
                qz_t = apool.tile([128, G // PACK], I32)
                nc.sync.dma_start(qz_t[:], qz_d[n0:n0 + 128, :])
                sc_t = apool.tile([128, G], F32)
                nc.sync.dma_start(sc_t[:], sc_d[n0:n0 + 128, :])

                zi_t = apool.tile([128, G], I32)
                for i in range(PACK):
                    nc.vector.tensor_scalar(
                        out=zi_t[:, i::PACK], in0=qz_t[:],
                        scalar1=4 * i, scalar2=0xF, op0=_LSR, op1=_AND)
                z_t = apool.tile([128, G], F32)
                nc.vector.tensor_copy(z_t[:], zi_t[:])
                # s256 = s * 256;  zs = -z * s * 256  (ACT scale/bias pair)
                s256_t = apool.tile([128, G], F32)
                nc.vector.tensor_scalar(
                    out=s256_t[:], in0=sc_t[:], scalar1=WSC, scalar2=0.0,
                    op0=_MUL, op1=_ADD)
                zs_t = apool.tile([128, G], F32)
                nc.vector.scalar_tensor_tensor(
                    out=zs_t[:], in0=z_t[:], scalar=-1.0, in1=s256_t[:],
                    op0=_MUL, op1=_MUL)

                w_t = wchpool.tile([128, K], BF16)
                for h in range(2):
                    w0 = h * (HALF // PACK)
                    nib_t = nibpool.tile([128, HALF], I32)
                    for i in range(PACK):
                        nc.vector.tensor_scalar(
                            out=nib_t[:, i::PACK],
                            in0=qw_t[:, w0:w0 + HALF // PACK],
                            scalar1=4 * i, scalar2=0xF, op0=_LSR, op1=_AND)
                    for gh in range(G // 2):
                        g = h * (G // 2) + gh
                        if ENG32[g] == 'a':
                            # ACT: out = nib * (s*256) + (-z*s*256)
                            nc.scalar.activation(
                                w_t[:, g * GS:(g + 1) * GS],
                                nib_t[:, gh * GS:(gh + 1) * GS],
                                IDENT, bias=zs_t[:, g:g + 1],
                                scale=s256_t[:, g:g + 1])
                        else:
                            # DVE: out = (nib - z) * (s*256)
                            nc.vector.tensor_scalar(
                                out=w_t[:, g * GS:(g + 1) * GS],
                                in0=nib_t[:, gh * GS:(gh + 1) * GS],
                                scalar1=z_t[:, g:g + 1],
                                scalar2=s256_t[:, g:g + 1],
                                op0=_SUB, op1=_MUL)

                # batched xbar transposes: w_t [128 n, 4096 k]
                #   bf16 chunks -> wT[:, :, n0:n0+128] ([128 kk, 26 c, 128 n])
                #   fp8 chunks: bf16 transpose to wtmp, then ACT cast
                nc.sync.dma_start_transpose(
                    wT[:, :, n0:n0 + 128], w_t[:, :KCB * 128])
                w8t_t = w8pool.tile([128, NFP8, 128], BF16)
                nc.sync.dma_start_transpose(w8t_t[:], w_t[:, KCB * 128:])
                nc.scalar.copy(wT8[:, :, n0:n0 + 128], w8t_t[:])

                # window 0, 128-col sub-blocks: as soon as chunk j of the
                # first window is transposed, run tokens 0-3 on its columns
                if j < 4:
                    for ti in sub0:
                        block(xts[ti], xqs[ti], ti, n0, 128)

                # rotation: window w's tokens each run one full n-block.
                # Loads are emitted with lookahead 2 so each mm's tile is
                # in flight while the previous block computes, and no
                # long-waiting load ever sits ahead of the drains' stores
                # on the gpsimd FIFO (the xT pool self-paces the stream).
                if j % 4 == 3:
                    w = j // 4
                    toks = rot[w]
                    for ti, _w in toks[:2]:
                        xts[ti] = load_x(ti)
                        xqs[ti] = load_xq(ti)
                    for idx, (ti, _w) in enumerate(toks):
                        block(xts[ti], xqs[ti], ti, w * MMN, MMN)
                        if idx + 2 < len(toks):
                            nx = toks[idx + 2][0]
                            xts[nx] = load_x(nx)
                            xqs[nx] = load_xq(nx)

            # ---- Phase B: stream all tokens' remaining n-blocks
            for ti in range(nt):
                blocks = remaining[ti]
                if not blocks:
                    continue
                xT_t = load_x(ti)
                xq_t = load_xq(ti)
                for nb in blocks:
                    block(xT_t, xq_t, ti, nb * MMN, MMN)

    nc.compile()
    return nc


_nc_cache = {}


def _get_nc(t_total: int = T):
    if t_total not in _nc_cache:
        _nc_cache[t_total] = build(t_total)
    return _nc_cache[t_total]


def _tile_x(x, t_total):
    # [T, K] f32 -> bf16 [nt, 128 kk, 32 c, 128 t] of x*2^4 so one chunk
    # is one contiguous DMA into the transposed SBUF layout
    nt = t_total // TCH
    xf = (x.reshape(-1, K)[:t_total] * np.float32(XSC)).astype(ml_dtypes.bfloat16)
    xt = xf.reshape(nt, TCH, KC, 128).transpose(0, 3, 2, 1)
    return np.ascontiguousarray(xt)


def kernel(x, qweight, qzeros, scales, bias, trace=False, t_total=T):
    xt = _tile_x(np.asarray(x, dtype=np.float32), t_total)
    in_maps = []
    for c in range(NCORES):
        sl = slice(c * NS, (c + 1) * NS)
        in_maps.append({
            "xt": xt,
            "qw": np.ascontiguousarray(qweight[sl]),
            "qz": np.ascontiguousarray(qzeros[sl]),
            "sc": np.ascontiguousarray(scales[sl]),
            "b": np.ascontiguousarray(bias[sl]),
        })
    nc = _get_nc(t_total)
    res = run_bass_kernel_spmd(nc, in_maps, core_ids=list(range(NCORES)),
                               trace=trace)
    out = np.concatenate([r["out"] for r in res.results], axis=1)
    if t_total == T:
        out = out.reshape(B, S, N)
    out = out.astype(np.float32, copy=False)
    if trace:
        return out, res
    return out
